# revision 5
# baseline (speedup 1.0000x reference)
"""Trainium2 Bass kernel for ItemEmbeddingLayer (embedding_lookup).

Reference computation:
    out = Q_matrix[items] @ skill_embedding[user]      # [8192, 128] f32

Sharding (the hint's data-parallel option): Q_matrix and the user's
embedding row are replicated; `items` is sharded batch-wise, 1024/core.

Per-core device kernel (v3 — dense table + SBUF lookup):
Per-item DMA gathers are descriptor-generation-bound on GpSimd SWDGE
(~8.6ns/row => ~9us/core minimum), so v3 avoids row-gather DMAs entirely:

  1. Load Q^T (bf16, [skill, item-vocab] layout) with big contiguous DMAs.
  2. Materialize the full item-embedding table QE^T = (Q @ emb)^T
     [128 k, 4096 vocab] f32 on the Tensor engine (emb stationary, Q^T
     moving, 512-col PSUM banks) — the classic "precompute the embedding
     table, then look up" restructuring.
  3. ap_gather (GpSimd compute, no DMA descriptors) looks up this core's
     1024 items from the SBUF-resident table: occ[k, i] = QE^T[k, items[i]].
  4. One contiguous 512KB DMA writes occ; the host unshards by
     transposing each core's [128, 1024] block to [1024, 128].
"""

import numpy as np
import ml_dtypes

import concourse.bass as bass
import concourse.bacc as bacc
import concourse.mybir as mybir
from concourse.tile import TileContext
from concourse.bass_utils import run_bass_kernel_spmd

N_CORES = 8
L = 8192            # total items (seq len)
LC = L // N_CORES   # items per core
S = 256             # skills
K = 128             # hidden
R = 4096            # Q_matrix rows (item vocab)
P = 128             # partitions
NB = R // 512       # 512-col matmul blocks (PSUM bank = 512 f32)
NQD = 4             # qt input DMA chunks (pipeline DMA vs matmul)


def build_bass() -> bass.Bass:
    nc = bacc.Bacc(trn_type="TRN2", dynamic_dma_scratch_size=16384)
    # qt[p, j, i] = Q[i, j*128 + p]   (Q^T in matmul rhs layout)
    qt = nc.declare_dram_parameter("qt", [P, 2, R], mybir.dt.bfloat16, isOutput=False)
    idx = nc.declare_dram_parameter("idx", [P, LC // 16], mybir.dt.int16, isOutput=False)
    # emb_t[p, j, k] = emb[j*128 + p, k]   (stationary lhsT layout)
    emb = nc.declare_dram_parameter("emb", [P, 2, K], mybir.dt.bfloat16, isOutput=False)
    # outT[k, i] = out[i, k]; host transposes back
    outT = nc.declare_dram_parameter("outT", [P, LC], mybir.dt.float32, isOutput=True)

    with (
        TileContext(nc) as tc,
        tc.tile_pool(name="main", bufs=1) as pool,
        tc.tile_pool(name="acc", bufs=8, space="PSUM") as apsum,
    ):
        idx_t = pool.tile([P, LC // 16], mybir.dt.int16)
        nc.sync.dma_start(out=idx_t[:], in_=idx[:])
        emb_t = pool.tile([P, 2, K], mybir.dt.bfloat16)
        nc.sync.dma_start(out=emb_t[:], in_=emb[:])

        qt_t = pool.tile([P, 2, R], mybir.dt.bfloat16)
        CH = R // NQD
        for c in range(NQD):
            # both skill-chunks for a vocab range; 2 strided spans per DMA
            nc.sync.dma_start(
                out=qt_t[:, :, c * CH : (c + 1) * CH],
                in_=qt[:, :, c * CH : (c + 1) * CH],
            )

        table = pool.tile([P, R], mybir.dt.float32)
        for n in range(NB):
            ps = apsum.tile([P, 512], mybir.dt.float32, tag="ps")
            for j in range(2):
                nc.tensor.matmul(
                    ps[:],
                    emb_t[:, j, :],
                    qt_t[:, j, n * 512 : (n + 1) * 512],
                    start=(j == 0),
                    stop=(j == 1),
                )
            # alternate copy engines so DVE and ACT split the PSUM drain
            if n % 2 == 0:
                nc.vector.tensor_copy(table[:, n * 512 : (n + 1) * 512], ps[:])
            else:
                nc.scalar.copy(table[:, n * 512 : (n + 1) * 512], ps[:])

        occ = pool.tile([P, LC], mybir.dt.float32)
        nc.gpsimd.ap_gather(
            out_ap=occ[:],
            in_ap=table[:],
            idxs_ap=idx_t[:],
            channels=P,
            num_elems=R,
            d=1,
            num_idxs=LC,
        )
        nc.sync.dma_start(out=outT[:], in_=occ[:])

    nc.compile()
    return nc


_CACHE: dict = {}


def get_nc() -> bass.Bass:
    if "nc" not in _CACHE:
        _CACHE["nc"] = build_bass()
    return _CACHE["nc"]


def make_in_maps(user, Q_matrix, items, skill_embedding):
    user = int(np.asarray(user))
    Q = np.asarray(Q_matrix, dtype=np.float32)
    items = np.asarray(items).astype(np.int64)
    emb32 = np.ascontiguousarray(np.asarray(skill_embedding)[user], dtype=np.float32)

    # Q^T in [p, j, i] rhs layout (bf16 exact: Q is 0/1)
    qt = np.ascontiguousarray(
        Q.T.reshape(2, P, R).transpose(1, 0, 2).astype(ml_dtypes.bfloat16)
    )
    emb_t = np.ascontiguousarray(
        emb32.reshape(2, P, K).transpose(1, 0, 2).astype(ml_dtypes.bfloat16)
    )

    in_maps = []
    for i in range(N_CORES):
        it = items[i * LC : (i + 1) * LC].astype(np.int16)
        # ap_gather idx i lives at [i % 16, i // 16], replicated to all
        # 8 groups of 16 partitions (one per GpSimd DSP core).
        blk = np.ascontiguousarray(it.reshape(LC // 16, 16).T)  # [16, LC//16]
        idx_arr = np.tile(blk, (8, 1))  # [128, LC//16]
        in_maps.append({"qt": qt, "idx": idx_arr, "emb": emb_t})
    return in_maps


def kernel(user, Q_matrix, items, skill_embedding, _trace=False, _result_box=None):
    in_maps = make_in_maps(user, Q_matrix, items, skill_embedding)
    res = run_bass_kernel_spmd(get_nc(), in_maps, list(range(N_CORES)), trace=_trace)
    if _result_box is not None:
        _result_box.append(res)
    out = np.concatenate(
        [np.ascontiguousarray(res.results[i]["outT"].T) for i in range(N_CORES)],
        axis=0,
    )
    return np.ascontiguousarray(out, dtype=np.float32)
